# revision 8
# baseline (speedup 1.0000x reference)
"""Trainium2 Bass kernel for nn_D2GroupConvolutionLayer (D2-equivariant GAT).

Math: for each output view g and input view h, the layer is a GAT with a
GLOBAL softmax over edges.  The edge score factorizes as
score(e) = u[src(e)] + v[dst(e)], so gather -> softmax -> scatter-add
collapses to dense algebra:

    out_gh = diag(b) . M . diag(a) . H / (b^T M a)

with a = exp(u - max u), b = exp(v - max v) per-node scalars and
M[d, s] = multiplicity of edge s->d (self-loops included) a fixed 0/1/2
matrix built on the host from edge_index.

Key optimizations vs a dense bf16 formulation:

- G = M @ (a*H) (the dominant 4 x 2048x2048x512 MACs/core) runs as fp8
  DoubleRow matmuls (2 k-tiles/pass, 0.5 cycles/row).  M's {0,1,2} are
  exact in fp8_e4m3; A = fp8(32*a*H) alone quantizes too coarsely
  (~3% -> rel err 3e-2), so a second DoubleRow pass adds the rounding
  residual fp8(32*a*H - A) into the same psum group: ~7 effective
  mantissa bits at 2x the bf16 MAC rate.  The 32 scale folded into a
  (exp(u-mu+ln32)) cancels exactly in G/z.
- z = b^T M a is computed BEFORE G via a tiny DoubleRow matvec
  (mt8 stationary, fp8 a moving -> M@a in [d-part, view] layout), so
  the psum drain applies diag(b/z) directly: no staging pass.
- u,v dot products: lrelu(x) = 0.2x + 0.8 relu(x); the linear term
  H@att rides the H matmul as 2 extra psum columns (wuv = W@att
  precomputed on host), and the relu term folds max(0,.) into the
  same DVE accumulate op.  exp() applies the 0.2 via its scale arg.
- Per-view drains run on ACT (psum * b/z -> fp16) and accumulate across
  views with cheap all-fp16 2x-rate DVE adds; output DMAs as fp16.

Sharding: data-parallel over the 8 (batch b, output view g) pairs, one
NeuronCore each; no communication.
"""

import math
import sys
from contextlib import ExitStack

for _p in ("/opt/trn_rl_repo/concourse", "/opt/trn_rl_repo"):
    if _p not in sys.path:
        sys.path.insert(0, _p)

import ml_dtypes  # noqa: E402
import numpy as np  # noqa: E402

import concourse.bass as bass  # noqa: E402
import concourse.bacc as bacc  # noqa: E402
import concourse.mybir as mybir  # noqa: E402
import concourse.tile as tile  # noqa: E402
import concourse.tile_utils as tile_utils  # noqa: E402
import bass_rust  # noqa: E402

# Problem constants (hardcoded per harness contract).
B, V, N, F, O = 2, 4, 2048, 128, 512
NT = N // 128  # node tiles
NP = NT // 2  # s-tile pairs for fp8 DoubleRow
NEG_SLOPE = 0.2
LN_SCALE = math.log(32.0)  # a is computed pre-scaled by 32 for fp8 range
F32 = mybir.dt.float32
BF16 = mybir.dt.bfloat16
FP16 = mybir.dt.float16
FP8 = mybir.dt.float8e4
DR = mybir.MatmulPerfMode.DoubleRow
MULT = mybir.AluOpType.mult
ADD = mybir.AluOpType.add
MAX = mybir.AluOpType.max
SUB = mybir.AluOpType.subtract

# Stock cap leaves 16KB/partition unused on trn2 (224 phys / 208 usable).
tile_utils.max_sbuf_usage = 204 * 1024


class _TileContext(tile.TileContext):
    """Splits the exit-drain's sem waits across single-wait carrier nops.

    Walrus caps sync waits at 1/instruction (2 for EventSemaphore); the stock
    _drain_and_barrier attaches every outstanding DMA/engine sem wait to one
    Drain and fails codegen with "Too many sync wait commands".
    """

    def _drain_and_barrier(self, tick_clock, wait_clock):
        nc = self.nc
        probe = nc.sync.nop(nofuse=True)
        wait_clock.add_sem_waits(
            probe.ins, bass_rust.ScopedClock({None: tick_clock.global_clock})
        )
        si = probe.ins.sync_info
        if si is not None and si.on_wait and len(si.on_wait) > 1:
            waits = list(si.on_wait)
            si.on_wait = [waits[0]]
            for w in waits[1:]:
                carrier = nc.sync.nop(nofuse=True)
                carrier.ins.sync_info = mybir.SyncInfo(on_wait=[w], on_update=[])
        nc.sync.drain()
        nc.all_engine_barrier()
        popped = nc._tile_sem_poison_stack.pop()
        assert popped is self._sem_poison
        nc.clear_and_free_semaphores(list(self.sems.allocated().values()))
        nc.all_engine_barrier()


def _build_program():
    nc = bacc.Bacc("TRN2", target_bir_lowering=False, debug=False)

    xpair_d = nc.dram_tensor("xpair", [V, 2, 128, N], BF16, kind="ExternalInput").ap()
    wsel_d = nc.dram_tensor("wsel", [V, 2, 128, O], BF16, kind="ExternalInput").ap()
    wuv_d = nc.dram_tensor("wuv", [V, 2, 128, 2], BF16, kind="ExternalInput").ap()
    mt8_d = nc.dram_tensor("mt8", [NT, 128, N], FP8, kind="ExternalInput").ap()
    attb_d = nc.dram_tensor("attb", [128, 2 * O], FP16, kind="ExternalInput").ap()
    out_d = nc.dram_tensor("out", [NT, 128, O], FP16, kind="ExternalOutput").ap()

    with ExitStack() as ctx:
        tc = ctx.enter_context(_TileContext(nc))
        pool = ctx.enter_context(tc.tile_pool(name="main", bufs=1))
        xpool = ctx.enter_context(tc.tile_pool(name="x", bufs=10))
        hpool = ctx.enter_context(tc.tile_pool(name="hg", bufs=2))
        h8pool = ctx.enter_context(tc.tile_pool(name="h8", bufs=2))
        l8pool = ctx.enter_context(tc.tile_pool(name="l8", bufs=2))
        spool = ctx.enter_context(tc.tile_pool(name="s", bufs=3))
        opool = ctx.enter_context(tc.tile_pool(name="o", bufs=4))
        stpool = ctx.enter_context(tc.tile_pool(name="st", bufs=2))
        # psum: ph/pg tiles (1 bank each) cycle a 6-slot ring; uv columns
        # get their own pool so their long-lived accumulation group never
        # shares a zero region with the ring.
        pp = ctx.enter_context(tc.tile_pool(name="ps", bufs=6, space="PSUM"))
        uvpool = ctx.enter_context(tc.tile_pool(name="uv", bufs=2, space="PSUM"))

        # ---- persistent SBUF tensors ----
        attb = pool.tile([128, 2 * O], FP16)
        wsel = pool.tile([128, V, 2, O], BF16)
        wuv = pool.tile([128, V, 2, 2], BF16)
        mt8 = pool.tile([128, NT, N], FP8)
        acc = pool.tile([128, NT, O], FP16)  # cross-view output accumulator
        a8 = pool.tile([128, NT, V], FP8)  # 32*a per view, fp8 for mavec
        ones = pool.tile([128, 1], F32)
        ones_row = pool.tile([1, 128], F32)
        zp = pool.tile([128, V], F32)
        z1 = pool.tile([1, V], F32)

        nc.sync.dma_start(attb[:], attb_d[:])
        for h in range(V):
            for i in range(2):
                nc.sync.dma_start(wsel[:, h, i, :], wsel_d[h, i])
                nc.sync.dma_start(wuv[:, h, i, :], wuv_d[h, i])

        nc.vector.memset(ones[:], 1.0)
        nc.vector.memset(ones_row[:], 1.0)

        st = {}
        _mrows = {}

        def mrow_t(h):
            if h not in _mrows:
                _mrows[h] = [
                    stpool.tile([1, 128], F32, tag=f"mr{j}", name=f"mr{h}_{j}")
                    for j in range(2)
                ]
            return _mrows[h]

        def h_mms(h, tiles):
            """H = x-pair @ W-pair into psum; ACT-copy to haug16 fp16.
            The u/v linear dot columns (H @ att halves) ride along as a
            [128, 2] psum window per tile via the wuv weights."""
            if tiles[0] == 0:
                xpc = []
                for i in range(2):
                    row = []
                    for c in range(4):
                        xc = xpool.tile([128, 512], BF16, tag="xp",
                                        name=f"xp{h}_{i}_{c}")
                        nc.sync.dma_start(
                            xc[:], xpair_d[h, i, :, c * 512: (c + 1) * 512]
                        )
                        row.append(xc)
                    xpc.append(row)
                haug = hpool.tile([128, NT, O], FP16, tag="haug", name=f"haug{h}")
                u_r = stpool.tile([128, NT], F32, tag="u", name=f"u{h}")
                v_r = stpool.tile([128, NT], F32, tag="v", name=f"v{h}")
                uvp = uvpool.tile([128, NT, 2], F32, tag="uv", name=f"uvp{h}")
                st[h] = [haug, u_r, v_r, xpc, uvp]
            haug, _, _, xpc, uvp = st[h][:5]
            for t in tiles:
                ph = pp.tile([128, O], F32, tag="ps", name=f"ph{h}_{t}")
                c, col = t // 4, (t % 4) * 128
                for i in range(2):
                    nc.tensor.matmul(
                        ph[:], xpc[i][c][:, col: col + 128], wsel[:, h, i, :],
                        start=(i == 0), stop=(i == 1),
                    )
                    # uv columns accumulate into one long-lived group that
                    # rides a single zero region (start/stop on first/last).
                    nc.tensor.matmul(
                        uvp[:, t, :], xpc[i][c][:, col: col + 128],
                        wuv[:, h, i, :],
                        start=(t == 0 and i == 0),
                        stop=(t == NT - 1 and i == 1),
                    )
                nc.scalar.copy(haug[:, t, :], ph[:])  # psum -> sbuf fp16

        def dots_t(h, t):
            """relu-part of the att dot products: relu folds into the
            accumulate op (lrelu = 0.2 x + 0.8 relu(x); 0.2 applied at exp)."""
            haug, u_r, v_r = st[h][:3]
            hb = haug[:, t, :]
            scr = spool.tile([128, O], FP16, tag="s", name=f"scru{h}_{t}")
            nc.vector.scalar_tensor_tensor(
                scr[:], hb, 0.0, attb[:, :O], op0=MAX, op1=MULT,
                accum_out=u_r[:, t: t + 1],
            )
            scr2 = spool.tile([128, O], FP16, tag="s", name=f"scrv{h}_{t}")
            nc.vector.scalar_tensor_tensor(
                scr2[:], hb, 0.0, attb[:, O:], op0=MAX, op1=MULT,
                accum_out=v_r[:, t: t + 1],
            )

        def stats_a(h):
            """u,v assembly + per-partition then global max (DMA transpose)."""
            _, u_r, v_r, _, uvp = st[h][:5]
            uvl = stpool.tile([128, NT, 2], F32, tag="uvl", name=f"uvl{h}")
            nc.vector.tensor_copy(uvl[:], uvp[:])
            u_all = stpool.tile([128, NT], F32, tag="ua", name=f"ua{h}")
            v_all = stpool.tile([128, NT], F32, tag="va", name=f"va{h}")
            # u = 0.2*(4*u_relu + u_lin); the 0.2 is exp's scale arg
            nc.vector.scalar_tensor_tensor(
                u_all[:], u_r[:], 4.0, uvl[:, :, 0], op0=MULT, op1=ADD,
            )
            nc.vector.scalar_tensor_tensor(
                v_all[:], v_r[:], 4.0, uvl[:, :, 1], op0=MULT, op1=ADD,
            )
            mstat = stpool.tile([128, 2], F32, tag="mst", name=f"mst{h}")
            st[h].extend([u_all, v_all, mstat])  # indices 5, 6, 7
            for j, stat in ((0, u_all), (1, v_all)):
                nc.vector.reduce_max(
                    mstat[:, j: j + 1], stat[:], axis=mybir.AxisListType.X
                )
                nc.sync.dma_start(mrow_t(h)[j][0:1, :], mstat[:, j: j + 1])

        def stats_b(h):
            """Global max -> negm; a32 = 32*exp(.2(u-mu)); b = exp(.2(v-mv))."""
            u_all, v_all = st[h][5], st[h][6]
            m1n = stpool.tile([1, 2], F32, tag="m1n", name=f"m1n{h}")
            negm = stpool.tile([128, 2], F32, tag="negm", name=f"negm{h}")
            for j in range(2):
                nc.vector.tensor_reduce(
                    m1n[0:1, j: j + 1], mrow_t(h)[j][0:1, :],
                    axis=mybir.AxisListType.X, op=MAX,
                )
            nc.scalar.mul(m1n[0:1, :], m1n[0:1, :], -NEG_SLOPE)
            nc.vector.tensor_scalar(
                m1n[0:1, 0:1], m1n[0:1, 0:1], LN_SCALE, None, op0=ADD,
            )
            pb = pp.tile([128, 2], F32, tag="ps", name=f"pbm{h}")
            nc.tensor.matmul(pb[:], ones_row[:], m1n[:], start=True, stop=True)
            nc.vector.tensor_copy(negm[:], pb[:])
            a32 = stpool.tile([128, NT], F32, tag="a32", name=f"a32{h}")
            b_st = stpool.tile([128, NT], F32, tag="bst", name=f"bst{h}")
            nc.scalar.activation(
                a32[:], u_all[:], mybir.ActivationFunctionType.Exp,
                bias=negm[:, 0:1], scale=NEG_SLOPE,
            )
            nc.scalar.activation(
                b_st[:], v_all[:], mybir.ActivationFunctionType.Exp,
                bias=negm[:, 1:2], scale=NEG_SLOPE,
            )
            nc.vector.tensor_copy(a8[:, :, h], a32[:])  # fp8 for mavec
            st[h].extend([a32, b_st])  # indices 8, 9

        def conv8(h, tiles):
            """h8 = fp8(a32*H) and its rounding residual lo8 = fp8(a32*H-h8).
            G accumulates M@h8 + M@lo8: ~7 effective mantissa bits at fp8
            DoubleRow speed."""
            if tiles[0] == 0:
                st[h].append(
                    h8pool.tile([128, NT, O], FP8, tag="h8", name=f"h8{h}")
                )  # index 10
                st[h].append(
                    l8pool.tile([128, NT, O], FP8, tag="l8", name=f"l8{h}")
                )  # index 11
            haug, a32, h8, lo8 = st[h][0], st[h][8], st[h][10], st[h][11]
            for t in tiles:
                nc.scalar.mul(h8[:, t, :], haug[:, t, :], a32[:, t: t + 1])
                nc.vector.scalar_tensor_tensor(
                    lo8[:, t, :], haug[:, t, :], a32[:, t: t + 1], h8[:, t, :],
                    op0=MULT, op1=SUB,
                )

        def mavec(h):
            """ma = M @ a32 via DoubleRow matvec (mt8 stationary, a8 moving):
            lands in [d-part, 1] psum windows, one zero region for all d."""
            ma_ps = pp.tile([128, NT], F32, tag="ps", name=f"maps{h}")
            for d in range(NT):
                for p in range(NP):
                    nc.tensor.matmul(
                        ma_ps[:, d: d + 1],
                        mt8[:, 2 * p: 2 * p + 2, d * 128: (d + 1) * 128],
                        a8[:, 2 * p: 2 * p + 2, h: h + 1],
                        start=(d == 0 and p == 0),
                        stop=(d == NT - 1 and p == NP - 1),
                        perf_mode=DR,
                    )
            st[h].append(ma_ps)  # index 12

        def z_chain(h):
            """rzb = b * 1/(V * b^T ma); gates only the drains of G(h)."""
            b_st, ma_ps = st[h][9], st[h][12]
            zscr = stpool.tile([128, NT], F32, tag="zscr", name=f"zscr{h}")
            nc.vector.scalar_tensor_tensor(
                zscr[:], ma_ps[:], 1.0, b_st[:], op0=MULT, op1=MULT,
                accum_out=zp[:, h: h + 1],
            )
            pzt = pp.tile([1, 1], F32, tag="ps", name=f"pz{h}")
            nc.tensor.matmul(
                pzt[:], ones[:], zp[:, h: h + 1], start=True, stop=True
            )
            nc.vector.reciprocal(z1[0:1, h: h + 1], pzt[:])
            nc.vector.tensor_scalar(
                z1[0:1, h: h + 1], z1[0:1, h: h + 1], 1.0 / V, None, op0=MULT,
            )
            przb = pp.tile([128, 1], F32, tag="ps", name=f"przb{h}")
            nc.tensor.matmul(
                przb[:], ones_row[:], z1[0:1, h: h + 1], start=True, stop=True
            )
            rzh = stpool.tile([128, 1], F32, tag="rz", name=f"rz{h}")
            nc.vector.tensor_copy(rzh[:], przb[:])
            rzb = stpool.tile([128, NT], F32, tag="rzb", name=f"rzb{h}")
            nc.vector.tensor_scalar(
                rzb[:], b_st[:], rzh[:, 0:1], None, op0=MULT,
            )
            st[h].append(rzb)  # index 13

        def g_tile(h, d):
            """G[d] = M @ (a*H) for view h via fp8 DoubleRow (hi + residual
            into one psum group); ACT applies b/z -> fp16, DVE accumulates
            across views at fp16 2x rate."""
            h8, lo8, rzb = st[h][10], st[h][11], st[h][13]
            HALF = O // 2
            pg = pp.tile([128, O], F32, tag="ps", name=f"pg{h}_{d}")
            for p in range(NP):
                lhsT = mt8[:, 2 * p: 2 * p + 2, d * 128: (d + 1) * 128]
                # all 4 matmuls ride one psum-bank zero region
                for k, src8 in enumerate((h8, lo8)):
                    nc.tensor.matmul(
                        pg[:, :HALF], lhsT,
                        src8[:, 2 * p: 2 * p + 2, :HALF],
                        start=(p == 0 and k == 0), stop=False, perf_mode=DR,
                    )
                    nc.tensor.matmul(
                        pg[:, HALF:], lhsT,
                        src8[:, 2 * p: 2 * p + 2, HALF:],
                        start=False, stop=(p == NP - 1 and k == 1),
                        perf_mode=DR,
                    )
            if h == 0:
                nc.scalar.mul(acc[:, d, :], pg[:], rzb[:, d: d + 1])
            else:
                o = opool.tile([128, O], FP16, tag="o", name=f"o{h}_{d}")
                nc.scalar.mul(o[:], pg[:], rzb[:, d: d + 1])
                nc.vector.tensor_tensor(
                    acc[:, d, :], acc[:, d, :], o[:], op=ADD,
                )
            if h == V - 1:
                nc.sync.dma_start(out_d[d], acc[:, d, :])

        # ---- pipeline ----
        # Lead-in: view 0 prepared alone (PE mostly idle, DVE/ACT-bound).
        h_mms(0, list(range(NT)))
        for t in range(NT):
            dots_t(0, t)
        # mt8 needed first by mavec(0); emit after view 0's x chunks.
        for s in range(NT):
            nc.sync.dma_start(mt8[:, s, :], mt8_d[s])
        stats_a(0)
        stats_b(0)
        conv8(0, list(range(NT)))
        mavec(0)
        z_chain(0)

        # Steady phases: G(h) overlaps preparation of view h+1.  stats and
        # the z chain are split across emission points so their serial
        # dependency chains never stall an otherwise-busy engine queue.
        for h in range(V):
            hn = h + 1 if h + 1 < V else None
            for d in range(NT):
                g_tile(h, d)
                if hn is not None:
                    if d < 8:
                        h_mms(hn, [2 * d, 2 * d + 1])
                        dots_t(hn, 2 * d)
                        dots_t(hn, 2 * d + 1)
                    elif d == 8:
                        stats_a(hn)
                    elif d == 10:
                        stats_b(hn)
                    elif d in (11, 12, 13):
                        conv8(hn, [5 * (d - 11) + k
                                   for k in range(5 if d < 13 else 6)])
                    elif d == 14:
                        mavec(hn)
                    elif d == 15:
                        z_chain(hn)

    nc.compile()
    return nc


_SIGNS = None


def _signs():
    global _SIGNS
    if _SIGNS is None:
        s = np.ones((4, F), dtype=np.float32)
        for r in range(4):
            if r & 1:
                s[r, [0, 2]] = -1.0
            if r & 2:
                s[r, [1, 3]] = -1.0
        _SIGNS = s
    return _SIGNS


def _host_prep(x, edge_index, W, att, bias):
    """Pure relayout/index preprocessing; no float math on tensor data
    beyond sign flips of W rows (exact +-1 scaling), dtype casts, and the
    tiny weights-only fold wuv = W_signed @ att ([128,512]@[512,2])."""
    signs = _signs()
    x = np.ascontiguousarray(x, dtype=np.float32)
    W = np.asarray(W, dtype=np.float32)
    att = np.asarray(att, dtype=np.float32).reshape(2 * O)
    ei = np.asarray(edge_index)

    # M^T tiles: mt8[s_tile][p, d] = M[d, s_tile*128 + p], fp8 (0/1/2 exact)
    M = np.zeros((N, N), dtype=np.float32)
    np.add.at(M, (ei[1], ei[0]), 1.0)
    M[np.arange(N), np.arange(N)] += 1.0
    MT = np.ascontiguousarray(M.T)
    mt8_tiles = np.ascontiguousarray(
        MT.reshape(NT, 128, N).astype(ml_dtypes.float8_e4m3)
    )

    W1, W2 = W[:F], W[F:]
    att_uv = np.stack([att[:O], att[O:]], axis=1)  # [O, 2]
    attb = np.ascontiguousarray(
        np.broadcast_to(att.reshape(1, 2 * O), (128, 2 * O))
    ).astype(np.float16)

    xT = np.ascontiguousarray(x.transpose(0, 1, 3, 2))  # [B, V, F, N]

    in_maps = []
    for core in range(8):
        b, g = divmod(core, V)
        xpair = np.empty((V, 2, 128, N), dtype=ml_dtypes.bfloat16)
        wselc = np.empty((V, 2, 128, O), dtype=ml_dtypes.bfloat16)
        wuvc = np.empty((V, 2, 128, 2), dtype=ml_dtypes.bfloat16)
        for h in range(V):
            xpair[h, 0] = xT[b, h]
            xpair[h, 1] = xT[b, g ^ h]
            w1s = signs[h ^ g][:, None] * W1
            w2s = signs[h][:, None] * W2
            wselc[h, 0] = w1s
            wselc[h, 1] = w2s
            wuvc[h, 0] = w1s @ att_uv
            wuvc[h, 1] = w2s @ att_uv
        in_maps.append(
            {
                "xpair": xpair,
                "wsel": wselc,
                "wuv": wuvc,
                "mt8": mt8_tiles,
                "attb": attb,
            }
        )
    return in_maps


_NC = None


def kernel(x, edge_index, W, att, bias):
    global _NC
    if _NC is None:
        _NC = _build_program()
    in_maps = _host_prep(x, edge_index, W, att, bias)

    from concourse.bass_utils import run_bass_kernel_spmd

    res = run_bass_kernel_spmd(_NC, in_maps, list(range(8)))
    bias_f = np.asarray(bias, dtype=np.float32)
    out = np.empty((B, V, N, O), dtype=np.float32)
    for core in range(8):
        b, g = divmod(core, V)
        out[b, g] = res.results[core]["out"].reshape(N, O).astype(np.float32)
    out += bias_f  # the layer's bias add; zeros in this problem's inputs
    return out


# revision 10
# speedup vs baseline: 1.0824x; 1.0824x over previous
"""Trainium2 Bass kernel for nn_D2GroupConvolutionLayer (D2-equivariant GAT).

Math: for each output view g and input view h, the layer is a GAT with a
GLOBAL softmax over edges.  The edge score factorizes as
score(e) = u[src(e)] + v[dst(e)], so gather -> softmax -> scatter-add
collapses to dense algebra:

    out_gh = diag(b) . M . diag(a) . H / (b^T M a)

with a = exp(u - max u), b = exp(v - max v) per-node scalars and
M[d, s] = multiplicity of edge s->d (self-loops included) a fixed 0/1/2
matrix built on the host from edge_index.

Key optimizations vs a dense bf16 formulation:

- G = M @ (a*H) (the dominant 4 x 2048x2048x512 MACs/core) runs as fp8
  DoubleRow matmuls (2 k-tiles/pass, 0.5 cycles/row).  M's {0,1,2} are
  exact in fp8_e4m3; A = fp8(32*a*H) alone quantizes too coarsely
  (~3% -> rel err 3e-2), so a second DoubleRow pass adds the rounding
  residual fp8(32*a*H - A) into the same psum group: ~7 effective
  mantissa bits at 2x the bf16 MAC rate.  The 32 scale folded into a
  (exp(u-mu+ln32)) cancels exactly in G/z.
- z = b^T M a is computed BEFORE G via a tiny DoubleRow matvec
  (mt8 stationary, fp8 a moving -> M@a in [d-part, view] layout), so
  the psum drain applies diag(b/z) directly: no staging pass.
- u,v dot products: lrelu(x) = 0.2x + 0.8 relu(x); the linear term
  H@att rides the H matmul as 2 extra psum columns (wuv = W@att
  precomputed on host), and the relu term folds max(0,.) into the
  same DVE accumulate op.  exp() applies the 0.2 via its scale arg.
- Per-view drains run on ACT (psum * b/z -> fp16) and accumulate across
  views with cheap all-fp16 2x-rate DVE adds; output DMAs as fp16.

Sharding: data-parallel over the 8 (batch b, output view g) pairs, one
NeuronCore each; no communication.
"""

import math
import sys
from contextlib import ExitStack

for _p in ("/opt/trn_rl_repo/concourse", "/opt/trn_rl_repo"):
    if _p not in sys.path:
        sys.path.insert(0, _p)

import ml_dtypes  # noqa: E402
import numpy as np  # noqa: E402

import concourse.bass as bass  # noqa: E402
import concourse.bacc as bacc  # noqa: E402
import concourse.mybir as mybir  # noqa: E402
import concourse.tile as tile  # noqa: E402
import concourse.tile_utils as tile_utils  # noqa: E402
import bass_rust  # noqa: E402

# Problem constants (hardcoded per harness contract).
B, V, N, F, O = 2, 4, 2048, 128, 512
NT = N // 128  # node tiles
NP = NT // 2  # s-tile pairs for fp8 DoubleRow
NEG_SLOPE = 0.2
LN_SCALE = math.log(32.0)  # a is computed pre-scaled by 32 for fp8 range
F32 = mybir.dt.float32
BF16 = mybir.dt.bfloat16
FP16 = mybir.dt.float16
FP8 = mybir.dt.float8e4
DR = mybir.MatmulPerfMode.DoubleRow
MULT = mybir.AluOpType.mult
ADD = mybir.AluOpType.add
MAX = mybir.AluOpType.max
SUB = mybir.AluOpType.subtract

# Stock cap leaves 16KB/partition unused on trn2 (224 phys / 208 usable).
tile_utils.max_sbuf_usage = 204 * 1024


class _TileContext(tile.TileContext):
    """Splits the exit-drain's sem waits across single-wait carrier nops.

    Walrus caps sync waits at 1/instruction (2 for EventSemaphore); the stock
    _drain_and_barrier attaches every outstanding DMA/engine sem wait to one
    Drain and fails codegen with "Too many sync wait commands".
    """

    def _drain_and_barrier(self, tick_clock, wait_clock):
        nc = self.nc
        probe = nc.sync.nop(nofuse=True)
        wait_clock.add_sem_waits(
            probe.ins, bass_rust.ScopedClock({None: tick_clock.global_clock})
        )
        si = probe.ins.sync_info
        if si is not None and si.on_wait and len(si.on_wait) > 1:
            waits = list(si.on_wait)
            si.on_wait = [waits[0]]
            for w in waits[1:]:
                carrier = nc.sync.nop(nofuse=True)
                carrier.ins.sync_info = mybir.SyncInfo(on_wait=[w], on_update=[])
        nc.sync.drain()
        nc.all_engine_barrier()
        popped = nc._tile_sem_poison_stack.pop()
        assert popped is self._sem_poison
        nc.clear_and_free_semaphores(list(self.sems.allocated().values()))
        nc.all_engine_barrier()


def _build_program():
    nc = bacc.Bacc("TRN2", target_bir_lowering=False, debug=False)

    xpair_d = nc.dram_tensor("xpair", [V, 2, 128, N], BF16, kind="ExternalInput").ap()
    wsel_d = nc.dram_tensor("wsel", [V, 2, 128, O], BF16, kind="ExternalInput").ap()
    wuv_d = nc.dram_tensor("wuv", [V, 2, 128, 2], BF16, kind="ExternalInput").ap()
    mt8_d = nc.dram_tensor("mt8", [NT, 128, N], FP8, kind="ExternalInput").ap()
    attb_d = nc.dram_tensor("attb", [128, 2 * O], FP16, kind="ExternalInput").ap()
    out_d = nc.dram_tensor("out", [NT, 128, O], FP16, kind="ExternalOutput").ap()

    with ExitStack() as ctx:
        tc = ctx.enter_context(_TileContext(nc))
        pool = ctx.enter_context(tc.tile_pool(name="main", bufs=1))
        xpool = ctx.enter_context(tc.tile_pool(name="x", bufs=16))
        hpool = ctx.enter_context(tc.tile_pool(name="hg", bufs=3))
        h8pool = ctx.enter_context(tc.tile_pool(name="h8", bufs=2))
        l8pool = ctx.enter_context(tc.tile_pool(name="l8", bufs=2))
        spool = ctx.enter_context(tc.tile_pool(name="s", bufs=3))
        opool = ctx.enter_context(tc.tile_pool(name="o", bufs=4))
        stpool = ctx.enter_context(tc.tile_pool(name="st", bufs=2))
        # psum: ph/pg tiles (1 bank each) cycle a 6-slot ring; uv columns
        # get their own pool so their long-lived accumulation group never
        # shares a zero region with the ring.
        pp = ctx.enter_context(tc.tile_pool(name="ps", bufs=6, space="PSUM"))
        uvpool = ctx.enter_context(tc.tile_pool(name="uv", bufs=2, space="PSUM"))

        # ---- persistent SBUF tensors ----
        attb = pool.tile([128, 2 * O], FP16)
        wsel = pool.tile([128, V, 2, O], BF16)
        wuv = pool.tile([128, V, 2, 2], BF16)
        mt8 = pool.tile([128, NT, N], FP8)
        acc = pool.tile([128, NT, O], FP16)  # cross-view output accumulator
        a8 = pool.tile([128, NT, V], FP8)  # 32*a per view, fp8 for mavec
        ones = pool.tile([128, 1], F32)
        ones_row = pool.tile([1, 128], F32)
        zp = pool.tile([128, V], F32)
        z1 = pool.tile([1, V], F32)

        nc.vector.memset(ones[:], 1.0)
        nc.vector.memset(ones_row[:], 1.0)

        st = {}
        _mrows = {}

        def mrow_t(h):
            if h not in _mrows:
                _mrows[h] = [
                    stpool.tile([1, 128], F32, tag=f"mr{j}", name=f"mr{h}_{j}")
                    for j in range(2)
                ]
            return _mrows[h]

        def h_mms(h, tiles):
            """H = x-pair @ W-pair into psum; ACT-copy to haug16 fp16.
            The u/v linear dot columns (H @ att halves) ride along as a
            [128, 2] psum window per tile via the wuv weights."""
            if tiles[0] == 0:
                xpc = [[None] * 4 for _ in range(2)]
                # first chunks + this view's weights lead the DMA queue so
                # the PE can start as early as possible
                for c in range(4):
                    for i in range(2):
                        xc = xpool.tile([128, 512], BF16, tag="xp",
                                        name=f"xp{h}_{i}_{c}")
                        nc.sync.dma_start(
                            xc[:], xpair_d[h, i, :, c * 512: (c + 1) * 512]
                        )
                        xpc[i][c] = xc
                        if h == 0 and c == 0 and i == 1:
                            for ii in range(2):
                                nc.sync.dma_start(
                                    wsel[:, 0, ii, :], wsel_d[0, ii])
                                nc.sync.dma_start(wuv[:, 0, ii, :], wuv_d[0, ii])
                haug = hpool.tile([128, NT, O], FP16, tag="haug", name=f"haug{h}")
                u_r = stpool.tile([128, NT], F32, tag="u", name=f"u{h}")
                v_r = stpool.tile([128, NT], F32, tag="v", name=f"v{h}")
                # padded to a full 2KB psum bank so the pool's two buffers
                # never share a zero region (start=True zeroes whole banks)
                uvp = uvpool.tile([128, NT, 32], F32, tag="uv", name=f"uvp{h}")
                st[h] = [haug, u_r, v_r, xpc, uvp]
            haug, _, _, xpc, uvp = st[h][:5]
            for t in tiles:
                ph = pp.tile([128, O], F32, tag="ps", name=f"ph{h}_{t}")
                c, col = t // 4, (t % 4) * 128
                for i in range(2):
                    nc.tensor.matmul(
                        ph[:], xpc[i][c][:, col: col + 128], wsel[:, h, i, :],
                        start=(i == 0), stop=(i == 1),
                    )
                    # uv columns accumulate into one long-lived group that
                    # rides a single zero region (start/stop on first/last).
                    nc.tensor.matmul(
                        uvp[:, t, 0:2], xpc[i][c][:, col: col + 128],
                        wuv[:, h, i, :],
                        start=(t == 0 and i == 0),
                        stop=(t == NT - 1 and i == 1),
                    )
                nc.scalar.copy(haug[:, t, :], ph[:])  # psum -> sbuf fp16

        def dots_t(h, t):
            """relu-part of the att dot products: relu folds into the
            accumulate op (lrelu = 0.2 x + 0.8 relu(x); 0.2 applied at exp)."""
            haug, u_r, v_r = st[h][:3]
            hb = haug[:, t, :]
            scr = spool.tile([128, O], FP16, tag="s", name=f"scru{h}_{t}")
            nc.vector.scalar_tensor_tensor(
                scr[:], hb, 0.0, attb[:, :O], op0=MAX, op1=MULT,
                accum_out=u_r[:, t: t + 1],
            )
            scr2 = spool.tile([128, O], FP16, tag="s", name=f"scrv{h}_{t}")
            nc.vector.scalar_tensor_tensor(
                scr2[:], hb, 0.0, attb[:, O:], op0=MAX, op1=MULT,
                accum_out=v_r[:, t: t + 1],
            )

        def stats_a(h):
            """u,v assembly + per-partition then global max (DMA transpose)."""
            _, u_r, v_r, _, uvp = st[h][:5]
            uvl = stpool.tile([128, NT, 2], F32, tag="uvl", name=f"uvl{h}")
            nc.vector.tensor_copy(uvl[:], uvp[:, :, 0:2])
            u_all = stpool.tile([128, NT], F32, tag="ua", name=f"ua{h}")
            v_all = stpool.tile([128, NT], F32, tag="va", name=f"va{h}")
            # u = 0.2*(4*u_relu + u_lin); the 0.2 is exp's scale arg
            nc.vector.scalar_tensor_tensor(
                u_all[:], u_r[:], 4.0, uvl[:, :, 0], op0=MULT, op1=ADD,
            )
            nc.vector.scalar_tensor_tensor(
                v_all[:], v_r[:], 4.0, uvl[:, :, 1], op0=MULT, op1=ADD,
            )
            mstat = stpool.tile([128, 2], F32, tag="mst", name=f"mst{h}")
            st[h].extend([u_all, v_all, mstat])  # indices 5, 6, 7
            for j, stat in ((0, u_all), (1, v_all)):
                nc.vector.reduce_max(
                    mstat[:, j: j + 1], stat[:], axis=mybir.AxisListType.X
                )
                nc.sync.dma_start(mrow_t(h)[j][0:1, :], mstat[:, j: j + 1])

        def stats_b(h):
            """Global max -> negm; a32 = 32*exp(.2(u-mu)); b = exp(.2(v-mv))."""
            u_all, v_all = st[h][5], st[h][6]
            m1n = stpool.tile([1, 2], F32, tag="m1n", name=f"m1n{h}")
            negm = stpool.tile([128, 2], F32, tag="negm", name=f"negm{h}")
            for j in range(2):
                nc.vector.tensor_reduce(
                    m1n[0:1, j: j + 1], mrow_t(h)[j][0:1, :],
                    axis=mybir.AxisListType.X, op=MAX,
                )
            nc.scalar.mul(m1n[0:1, :], m1n[0:1, :], -NEG_SLOPE)
            nc.vector.tensor_scalar(
                m1n[0:1, 0:1], m1n[0:1, 0:1], LN_SCALE, None, op0=ADD,
            )
            pb = pp.tile([128, 2], F32, tag="ps", name=f"pbm{h}")
            nc.tensor.matmul(pb[:], ones_row[:], m1n[:], start=True, stop=True)
            nc.vector.tensor_copy(negm[:], pb[:])
            a32 = stpool.tile([128, NT], F32, tag="a32", name=f"a32{h}")
            b_st = stpool.tile([128, NT], F32, tag="bst", name=f"bst{h}")
            nc.scalar.activation(
                a32[:], u_all[:], mybir.ActivationFunctionType.Exp,
                bias=negm[:, 0:1], scale=NEG_SLOPE,
            )
            nc.scalar.activation(
                b_st[:], v_all[:], mybir.ActivationFunctionType.Exp,
                bias=negm[:, 1:2], scale=NEG_SLOPE,
            )
            nc.vector.tensor_copy(a8[:, :, h], a32[:])  # fp8 for mavec
            st[h].extend([a32, b_st])  # indices 8, 9

        def conv8(h, tiles):
            """h8 = fp8(a32*H) and its rounding residual lo8 = fp8(a32*H-h8).
            G accumulates M@h8 + M@lo8: ~7 effective mantissa bits at fp8
            DoubleRow speed."""
            if tiles[0] == 0:
                st[h].append(
                    h8pool.tile([128, NT, O], FP8, tag="h8", name=f"h8{h}")
                )  # index 10
                st[h].append(
                    l8pool.tile([128, NT, O], FP8, tag="l8", name=f"l8{h}")
                )  # index 11
            haug, a32, h8, lo8 = st[h][0], st[h][8], st[h][10], st[h][11]
            for t in tiles:
                nc.scalar.mul(h8[:, t, :], haug[:, t, :], a32[:, t: t + 1])
                nc.vector.scalar_tensor_tensor(
                    lo8[:, t, :], haug[:, t, :], a32[:, t: t + 1], h8[:, t, :],
                    op0=MULT, op1=SUB,
                )

        def mavec(h):
            """ma = M @ a32 via DoubleRow matvec (mt8 stationary, a8 moving):
            lands in [d-part, 1] psum windows, one zero region for all d."""
            ma_ps = pp.tile([128, NT], F32, tag="ps", name=f"maps{h}")
            for d in range(NT):
                for p in range(NP):
                    nc.tensor.matmul(
                        ma_ps[:, d: d + 1],
                        mt8[:, 2 * p: 2 * p + 2, d * 128: (d + 1) * 128],
                        a8[:, 2 * p: 2 * p + 2, h: h + 1],
                        start=(d == 0 and p == 0),
                        stop=(d == NT - 1 and p == NP - 1),
                        perf_mode=DR,
                    )
            st[h].append(ma_ps)  # index 12

        def z_chain(h):
            """rzb = b * 1/(V * b^T ma); gates only the drains of G(h)."""
            b_st, ma_ps = st[h][9], st[h][12]
            zscr = stpool.tile([128, NT], F32, tag="zscr", name=f"zscr{h}")
            nc.vector.scalar_tensor_tensor(
                zscr[:], ma_ps[:], 1.0, b_st[:], op0=MULT, op1=MULT,
                accum_out=zp[:, h: h + 1],
            )
            pzt = pp.tile([1, 1], F32, tag="ps", name=f"pz{h}")
            nc.tensor.matmul(
                pzt[:], ones[:], zp[:, h: h + 1], start=True, stop=True
            )
            nc.vector.reciprocal(z1[0:1, h: h + 1], pzt[:])
            nc.vector.tensor_scalar(
                z1[0:1, h: h + 1], z1[0:1, h: h + 1], 1.0 / V, None, op0=MULT,
            )
            przb = pp.tile([128, 1], F32, tag="ps", name=f"przb{h}")
            nc.tensor.matmul(
                przb[:], ones_row[:], z1[0:1, h: h + 1], start=True, stop=True
            )
            rzh = stpool.tile([128, 1], F32, tag="rz", name=f"rz{h}")
            nc.vector.tensor_copy(rzh[:], przb[:])
            rzb = stpool.tile([128, NT], F32, tag="rzb", name=f"rzb{h}")
            nc.vector.tensor_scalar(
                rzb[:], b_st[:], rzh[:, 0:1], None, op0=MULT,
            )
            st[h].append(rzb)  # index 13

        def g_tile(h, d):
            """G[d] = M @ (a*H) for view h via fp8 DoubleRow (hi + residual
            into one psum group); ACT applies b/z -> fp16, DVE accumulates
            across views at fp16 2x rate."""
            h8, lo8, rzb = st[h][10], st[h][11], st[h][13]
            HALF = O // 2
            pg = pp.tile([128, O], F32, tag="ps", name=f"pg{h}_{d}")
            for p in range(NP):
                lhsT = mt8[:, 2 * p: 2 * p + 2, d * 128: (d + 1) * 128]
                # all 4 matmuls ride one psum-bank zero region
                for k, src8 in enumerate((h8, lo8)):
                    nc.tensor.matmul(
                        pg[:, :HALF], lhsT,
                        src8[:, 2 * p: 2 * p + 2, :HALF],
                        start=(p == 0 and k == 0), stop=False, perf_mode=DR,
                    )
                    nc.tensor.matmul(
                        pg[:, HALF:], lhsT,
                        src8[:, 2 * p: 2 * p + 2, HALF:],
                        start=False, stop=(p == NP - 1 and k == 1),
                        perf_mode=DR,
                    )
            if h == 0:
                nc.scalar.mul(acc[:, d, :], pg[:], rzb[:, d: d + 1])
            else:
                o = opool.tile([128, O], FP16, tag="o", name=f"o{h}_{d}")
                nc.scalar.mul(o[:], pg[:], rzb[:, d: d + 1])
                nc.vector.tensor_tensor(
                    acc[:, d, :], acc[:, d, :], o[:], op=ADD,
                )
            if h == V - 1:
                nc.sync.dma_start(out_d[d], acc[:, d, :])

        # ---- pipeline ----
        # Lead-in: view 0's H+dots+stats+conv8; view 1's H fills the PE
        # while view 0's elementwise chain runs on DVE/ACT.
        h_mms(0, list(range(NT)))
        nc.sync.dma_start(attb[:], attb_d[:])
        for t in range(NT):
            dots_t(0, t)
        h_mms(1, list(range(NT)))
        for hh in range(1, V):
            for ii in range(2):
                nc.sync.dma_start(wsel[:, hh, ii, :], wsel_d[hh, ii])
                nc.sync.dma_start(wuv[:, hh, ii, :], wuv_d[hh, ii])
        # mt8 needed first by mavec(0)
        for s in range(NT):
            nc.sync.dma_start(mt8[:, s, :], mt8_d[s])
        stats_a(0)
        stats_b(0)
        conv8(0, list(range(NT)))
        mavec(0)
        z_chain(0)

        # Steady phases: G(h) overlaps dots/stats/conv8 of view h+1 and the
        # H matmuls of view h+2.  stats and the z chain are split across
        # emission points so their serial dependency chains never stall an
        # otherwise-busy engine queue.
        for h in range(V):
            hn = h + 1 if h + 1 < V else None
            hn2 = h + 2 if h + 2 < V else None
            for d in range(NT):
                g_tile(h, d)
                if hn is not None:
                    if d < 8:
                        dots_t(hn, 2 * d)
                        dots_t(hn, 2 * d + 1)
                        if hn2 is not None:
                            h_mms(hn2, [2 * d, 2 * d + 1])
                    elif d == 8:
                        stats_a(hn)
                    elif d == 10:
                        stats_b(hn)
                    elif d in (11, 12, 13):
                        conv8(hn, [5 * (d - 11) + k
                                   for k in range(5 if d < 13 else 6)])
                    elif d == 14:
                        mavec(hn)
                    elif d == 15:
                        z_chain(hn)

    nc.compile()
    return nc


_SIGNS = None


def _signs():
    global _SIGNS
    if _SIGNS is None:
        s = np.ones((4, F), dtype=np.float32)
        for r in range(4):
            if r & 1:
                s[r, [0, 2]] = -1.0
            if r & 2:
                s[r, [1, 3]] = -1.0
        _SIGNS = s
    return _SIGNS


def _host_prep(x, edge_index, W, att, bias):
    """Pure relayout/index preprocessing; no float math on tensor data
    beyond sign flips of W rows (exact +-1 scaling), dtype casts, and the
    tiny weights-only fold wuv = W_signed @ att ([128,512]@[512,2])."""
    signs = _signs()
    x = np.ascontiguousarray(x, dtype=np.float32)
    W = np.asarray(W, dtype=np.float32)
    att = np.asarray(att, dtype=np.float32).reshape(2 * O)
    ei = np.asarray(edge_index)

    # M^T tiles: mt8[s_tile][p, d] = M[d, s_tile*128 + p], fp8 (0/1/2 exact)
    M = np.zeros((N, N), dtype=np.float32)
    np.add.at(M, (ei[1], ei[0]), 1.0)
    M[np.arange(N), np.arange(N)] += 1.0
    MT = np.ascontiguousarray(M.T)
    mt8_tiles = np.ascontiguousarray(
        MT.reshape(NT, 128, N).astype(ml_dtypes.float8_e4m3)
    )

    W1, W2 = W[:F], W[F:]
    att_uv = np.stack([att[:O], att[O:]], axis=1)  # [O, 2]
    attb = np.ascontiguousarray(
        np.broadcast_to(att.reshape(1, 2 * O), (128, 2 * O))
    ).astype(np.float16)

    xT = np.ascontiguousarray(x.transpose(0, 1, 3, 2))  # [B, V, F, N]

    in_maps = []
    for core in range(8):
        b, g = divmod(core, V)
        xpair = np.empty((V, 2, 128, N), dtype=ml_dtypes.bfloat16)
        wselc = np.empty((V, 2, 128, O), dtype=ml_dtypes.bfloat16)
        wuvc = np.empty((V, 2, 128, 2), dtype=ml_dtypes.bfloat16)
        for h in range(V):
            xpair[h, 0] = xT[b, h]
            xpair[h, 1] = xT[b, g ^ h]
            w1s = signs[h ^ g][:, None] * W1
            w2s = signs[h][:, None] * W2
            wselc[h, 0] = w1s
            wselc[h, 1] = w2s
            wuvc[h, 0] = w1s @ att_uv
            wuvc[h, 1] = w2s @ att_uv
        in_maps.append(
            {
                "xpair": xpair,
                "wsel": wselc,
                "wuv": wuvc,
                "mt8": mt8_tiles,
                "attb": attb,
            }
        )
    return in_maps


_NC = None


def kernel(x, edge_index, W, att, bias):
    global _NC
    if _NC is None:
        _NC = _build_program()
    in_maps = _host_prep(x, edge_index, W, att, bias)

    from concourse.bass_utils import run_bass_kernel_spmd

    res = run_bass_kernel_spmd(_NC, in_maps, list(range(8)))
    bias_f = np.asarray(bias, dtype=np.float32)
    out = np.empty((B, V, N, O), dtype=np.float32)
    for core in range(8):
        b, g = divmod(core, V)
        out[b, g] = res.results[core]["out"].reshape(N, O).astype(np.float32)
    out += bias_f  # the layer's bias add; zeros in this problem's inputs
    return out
